# revision 1
# baseline (speedup 1.0000x reference)
"""Trainium2 Bass kernel for CGRCNet-style cold-item scoring.

Computes, for U=2048 users and C=1024 cold items:
    x        = item_content @ Wi.T + bi          (only the cold rows are needed)
    xc       = x[cold_ids]                        (C, D)
    hu       = h_u_bar @ W1h.T                    (U, H)
    hx       = xc @ W1x.T                         (C, H)
    logits   = einsum('uch,h->uc', relu(hu[:,None,:] + hx[None,:,:] + b1), W2[0]) + b2

Sharding: U across 8 cores (256 users/core); everything else replicated.
The cold-row gather (zero FLOPs) happens on the host as part of input
distribution; all matrix math runs on device.

Device-side plan (per core), layouts are transposed so H lives on partitions:
  stage 1: xcT  (D=64p,  C=1024f) = WiT.T @ xcgT   (K=300 in 3 chunks) + bi
  stage 2: hxbT (H=128p, C=1024f) = W1xT.T @ xcT + b1
  stage 3: huT  (H=128p, U=256f)  = W1hT.T @ huT_in
  main loop over users u:
     R_u = relu(hxbT + huT[:,u])    one fused op (DVE tensor_scalar add+max,
                                    or ACT activation(Relu, bias=) for a split)
     logits[u, :] = W2 . R_u        PE matvec; the stationary operand is a
                                    (128, 32) slice of "w2big" whose single
                                    nonzero column selects the PSUM partition,
                                    so 32 users accumulate into one col-group
                                    and 128 users pack densely into one bank.
  evict PSUM bank (+b2) -> SBUF -> DMA to HBM.
"""

import os
import numpy as np

# ---------------- problem constants (hardcoded per contract) ----------------
U, D = 2048, 64
I_ITEMS, CD = 50000, 300
C = 1024
H = 128
N_CORES = 8
UL = U // N_CORES            # 256 users per core
COHORT = 128                 # users per PSUM-bank pair
WAVES = 32                   # users per col-group (accumulation depth)
GROUPS = 4                   # PE col groups (32 partitions each)
HALF = 512                   # free-dim half (PSUM bank = 512 fp32)

# ---------------- tunables ----------------
# dtype of the stored hxbT operand ("f32" accurate / "f16" fast DVE 4x mode)
HXB_DT = os.environ.get("KRN_HXB_DT", "f16")
# dtype of the relu output R / matvec operands ("f16" -> 1cyc/row PE; "f32")
R_DT = os.environ.get("KRN_R_DT", "f16")
# when R_DT == "f32": bitcast matvec operands to float32r (1 cyc/row on PE)
MV_F32R = os.environ.get("KRN_MV_F32R", "1") == "1"
# fraction of users whose elementwise op runs on ACT instead of DVE: num/den
ACT_NUM = int(os.environ.get("KRN_ACT_NUM", "7"))
ACT_DEN = 32
# engine for PSUM->SBUF evictions: "act" or "dve"
EV_ENG = os.environ.get("KRN_EV_ENG", "dve")
# dense PSUM packing: interleave all 4 col groups in one bank pair per cohort
# (zero-weight init wave makes has_written semantics robust); f16 path only
DENSE_EV = os.environ.get("KRN_DENSE_EV", "0") == "1"
# stage-2/3 matmuls via float32r bitcast (faster, but exercises f32r codegen)
STG_F32R = os.environ.get("KRN_STG_F32R", "0") == "1"

_CACHE = {}


def _dt(mybir, s):
    return {"f32": mybir.dt.float32, "f16": mybir.dt.float16}[s]


def build_bass(reps=1, hxb_dts=None, r_dts=None, mv_f32r=None, act_num=None,
               ev_eng=None, dense_ev=None):
    """Build + compile the SPMD single-core program. Returns the Bacc object.

    reps>1 repeats the whole body (benchmarking aid: wall-clock slope vs reps
    isolates device exec time from dispatch overhead)."""
    HXB_DT = hxb_dts if hxb_dts is not None else globals()["HXB_DT"]
    R_DT = r_dts if r_dts is not None else globals()["R_DT"]
    MV_F32R = mv_f32r if mv_f32r is not None else globals()["MV_F32R"]
    ACT_NUM = act_num if act_num is not None else globals()["ACT_NUM"]
    EV_ENG = ev_eng if ev_eng is not None else globals()["EV_ENG"]
    DENSE_EV = dense_ev if dense_ev is not None else globals()["DENSE_EV"]
    if DENSE_EV:
        assert R_DT == "f16", "dense eviction implemented for f16 path only"
    key = (HXB_DT, R_DT, MV_F32R, ACT_NUM, EV_ENG, DENSE_EV, STG_F32R, reps)
    if key in _CACHE:
        return _CACHE[key]

    import concourse.bacc as bacc
    import concourse.mybir as mybir
    from concourse import tile

    F32 = mybir.dt.float32
    hxb_dt = _dt(mybir, HXB_DT)
    r_dt = _dt(mybir, R_DT)
    w2_dt = r_dt
    ADD = mybir.AluOpType.add
    MAX = mybir.AluOpType.max
    RELU = mybir.ActivationFunctionType.Relu

    nc = bacc.Bacc("TRN2", target_bir_lowering=False, debug=False,
                   num_devices=N_CORES)

    # ---- DRAM tensors (names are the in_map keys) ----
    xcgT_d = nc.dram_tensor("xcgT", [CD, C], mybir.dt.float16,
                            kind="ExternalInput").ap()
    wiT_d = nc.dram_tensor("wiT", [CD, D], mybir.dt.float16,
                           kind="ExternalInput").ap()
    bicol_d = nc.dram_tensor("bicol", [D, 1], F32, kind="ExternalInput").ap()
    w1xT_d = nc.dram_tensor("w1xT", [D, H], F32, kind="ExternalInput").ap()
    w1hT_d = nc.dram_tensor("w1hT", [D, H], F32, kind="ExternalInput").ap()
    b1col_d = nc.dram_tensor("b1col", [H, 1], F32, kind="ExternalInput").ap()
    huT_d = nc.dram_tensor("huT", [D, UL], F32, kind="ExternalInput").ap()
    w2big_d = nc.dram_tensor("w2big", [H, (WAVES + 1) * 32], w2_dt,
                             kind="ExternalInput").ap()
    b2col_d = nc.dram_tensor("b2col", [H, 1], F32, kind="ExternalInput").ap()
    logits_d = nc.dram_tensor("logits", [UL, C], F32, kind="ExternalOutput").ap()

    KCH = [(0, 128), (128, 128), (256, CD - 256)]  # K chunks of the CD=300 dim

    with tile.TileContext(nc) as tc:
        with (
            tc.tile_pool(name="const", bufs=1) as constp,
            tc.tile_pool(name="work", bufs=1) as workp,
            tc.tile_pool(name="rpool", bufs=8) as rpool,
            tc.tile_pool(name="evpool", bufs=4) as evpool,
        ):
            for rep in range(reps):
                # ---- load replicated operands ----
                xcgT_sb = []
                wiT_sb = []
                for i, (k0, kn) in enumerate(KCH):
                    t = constp.tile([kn, C], mybir.dt.float16, name=f"rep{rep}_xcgT_sb{i}", tag=f"xcg{i}")
                    nc.sync.dma_start(t[:, :], xcgT_d[k0:k0 + kn, :])
                    xcgT_sb.append(t)
                    w = constp.tile([kn, D], mybir.dt.float16, name=f"rep{rep}_wiT_sb{i}", tag=f"wiT{i}")
                    nc.sync.dma_start(w[:, :], wiT_d[k0:k0 + kn, :])
                    wiT_sb.append(w)
                w1xT_sb = constp.tile([D, H], F32, name=f"rep{rep}_w1xT_sb", tag="w1xT")
                nc.sync.dma_start(w1xT_sb[:, :], w1xT_d[:, :])
                w1hT_sb = constp.tile([D, H], F32, name=f"rep{rep}_w1hT_sb", tag="w1hT")
                nc.sync.dma_start(w1hT_sb[:, :], w1hT_d[:, :])
                huTin_sb = constp.tile([D, UL], F32, name=f"rep{rep}_huTin_sb", tag="huTin")
                nc.sync.dma_start(huTin_sb[:, :], huT_d[:, :])
                b1col_sb = constp.tile([H, 1], F32, name=f"rep{rep}_b1col_sb", tag="b1col")
                nc.sync.dma_start(b1col_sb[:, :], b1col_d[:, :])
                bicol_sb = constp.tile([D, 1], F32, name=f"rep{rep}_bicol_sb", tag="bicol")
                nc.sync.dma_start(bicol_sb[:, :], bicol_d[:, :])
                b2col_sb = constp.tile([H, 1], F32, name=f"rep{rep}_b2col_sb", tag="b2col")
                nc.sync.dma_start(b2col_sb[:, :], b2col_d[:, :])
                w2big_sb = constp.tile([H, (WAVES + 1) * 32], w2_dt, name=f"rep{rep}_w2big_sb", tag="w2big")
                nc.sync.dma_start(w2big_sb[:, :], w2big_d[:, :])

                # ---- stages use a small psum pool that is released before the
                # main loop (PSUM budget: 8 banks total) ----
                xcT_sb = workp.tile([D, C], F32, name=f"rep{rep}_xcT_sb", tag="xcT")
                hxbT_sb = workp.tile([H, C], hxb_dt, name=f"rep{rep}_hxbT_sb", tag="hxbT")
                huT_sb = workp.tile([H, UL], F32, name=f"rep{rep}_huT_sb", tag="huT")
                with tc.tile_pool(name=f"rep{rep}_pstg", bufs=2, space="PSUM") as pstg:
                    # stage 1: xcT (64, 1024) = WiT.T @ xcgT + bi
                    for n in range(2):
                        ps1 = pstg.tile([H, HALF], F32, name=f"rep{rep}_ps_s1_{n}",
                                        tag="pstg")
                        for k, (k0, kn) in enumerate(KCH):
                            nc.tensor.matmul(
                                ps1[0:D, :], wiT_sb[k][:, :],
                                xcgT_sb[k][:, n * HALF:(n + 1) * HALF],
                                start=(k == 0), stop=(k == len(KCH) - 1),
                            )
                        if EV_ENG == "act":
                            nc.scalar.activation(
                                xcT_sb[:, n * HALF:(n + 1) * HALF], ps1[0:D, :],
                                mybir.ActivationFunctionType.Identity,
                                bias=bicol_sb[:, 0:1], scale=1.0)
                        else:
                            nc.vector.tensor_scalar(
                                xcT_sb[:, n * HALF:(n + 1) * HALF], ps1[0:D, :],
                                bicol_sb[:, 0:1], None, ADD)

                    # stage 2: hxbT (128, 1024) = W1xT.T @ xcT + b1
                    for n in range(2):
                        ps2 = pstg.tile([H, HALF], F32, name=f"rep{rep}_ps_s2_{n}",
                                        tag="pstg")
                        if STG_F32R:
                            nc.tensor.matmul(
                                ps2[:, :],
                                w1xT_sb[:, :].bitcast(mybir.dt.float32r),
                                xcT_sb[:, n * HALF:(n + 1) * HALF].bitcast(
                                    mybir.dt.float32r),
                                start=True, stop=True)
                        else:
                            nc.tensor.matmul(
                                ps2[:, :], w1xT_sb[:, :],
                                xcT_sb[:, n * HALF:(n + 1) * HALF],
                                start=True, stop=True)
                        if EV_ENG == "act":
                            nc.scalar.activation(
                                hxbT_sb[:, n * HALF:(n + 1) * HALF], ps2[:, :],
                                mybir.ActivationFunctionType.Identity,
                                bias=b1col_sb[:, 0:1], scale=1.0)
                        else:
                            nc.vector.tensor_scalar(
                                hxbT_sb[:, n * HALF:(n + 1) * HALF], ps2[:, :],
                                b1col_sb[:, 0:1], None, ADD)

                    # stage 3: huT (128, 256) = W1hT.T @ huT_in
                    ps3 = pstg.tile([H, HALF], F32, name=f"rep{rep}_ps_s3", tag="pstg")
                    if STG_F32R:
                        nc.tensor.matmul(
                            ps3[:, 0:UL],
                            w1hT_sb[:, :].bitcast(mybir.dt.float32r),
                            huTin_sb[:, :].bitcast(mybir.dt.float32r),
                            start=True, stop=True)
                    else:
                        nc.tensor.matmul(
                            ps3[:, 0:UL], w1hT_sb[:, :], huTin_sb[:, :],
                            start=True, stop=True)
                    if EV_ENG == "act":
                        nc.scalar.copy(huT_sb[:, :], ps3[:, 0:UL])
                    else:
                        nc.vector.tensor_copy(huT_sb[:, :], ps3[:, 0:UL])

                # ---- main loop ----
                # Each (col-group j, half h) owns a full PSUM bank; group j's 32
                # users accumulate into partitions [32j, 32j+32) of its bank via
                # the shifted-column stationary operand. One accumulation group
                # per bank -> well-defined has_written semantics.
                n_cohorts = UL // COHORT
                if DENSE_EV:
                    with tc.tile_pool(name=f"rep{rep}_plogd", bufs=4,
                                      space="PSUM") as plog:
                        zsl = w2big_sb[:, WAVES * 32:WAVES * 32 + 32]
                        for co in range(n_cohorts):
                            pbank = [plog.tile([H, HALF], F32,
                                               name=f"rep{rep}_plogd_{co}_{h}",
                                               tag="plogd") for h in range(2)]
                            for j in range(GROUPS):
                                for h in range(2):
                                    nc.tensor.matmul(
                                        pbank[h][32 * j:32 * j + 32, :], zsl,
                                        hxbT_sb[:, h * HALF:(h + 1) * HALF],
                                        start=True, stop=False,
                                        tile_position=(0, 32 * j),
                                        skip_group_check=True)
                            for r in range(WAVES):
                                for j in range(GROUPS):
                                    ul = 32 * j + r
                                    u = co * COHORT + ul
                                    rt = rpool.tile([H, C], r_dt,
                                                    name=f"rep{rep}_R_{u}",
                                                    tag="R")
                                    if (ul * ACT_NUM) % ACT_DEN < ACT_NUM:
                                        nc.scalar.activation(
                                            rt[:, :], hxbT_sb[:, :], RELU,
                                            bias=huT_sb[:, u:u + 1], scale=1.0)
                                    else:
                                        nc.vector.tensor_scalar(
                                            rt[:, :], hxbT_sb[:, :],
                                            huT_sb[:, u:u + 1], 0.0, ADD, MAX)
                                    lhsT = w2big_sb[:, 32 * r:32 * r + 32]
                                    for h in range(2):
                                        nc.tensor.matmul(
                                            pbank[h][32 * j:32 * j + 32, :],
                                            lhsT,
                                            rt[:, h * HALF:(h + 1) * HALF],
                                            start=False,
                                            stop=(r == WAVES - 1),
                                            tile_position=(0, 32 * j),
                                            skip_group_check=True)
                            for h in range(2):
                                ev = evpool.tile([H, HALF], F32,
                                                 name=f"rep{rep}_evd_{co}_{h}",
                                                 tag="ev")
                                if EV_ENG == "act":
                                    nc.scalar.activation(
                                        ev[:, :], pbank[h][:, :],
                                        mybir.ActivationFunctionType.Identity,
                                        bias=b2col_sb[:, 0:1], scale=1.0)
                                else:
                                    nc.vector.tensor_scalar(
                                        ev[:, :], pbank[h][:, :],
                                        b2col_sb[:, 0:1], None, ADD)
                                nc.sync.dma_start(
                                    logits_d[co * COHORT:(co + 1) * COHORT,
                                             h * HALF:(h + 1) * HALF],
                                    ev[:, :])
                else:
                    with tc.tile_pool(name=f"rep{rep}_plog", bufs=8, space="PSUM") as plog:
                        for co in range(n_cohorts):
                            pbank = [[plog.tile([H, HALF], F32,
                                                name=f"rep{rep}_plog_{co}_{j}_{h}", tag="plog")
                                      for h in range(2)] for j in range(GROUPS)]
                            for r in range(WAVES):
                                for j in range(GROUPS):
                                    ul = 32 * j + r          # partition within cohort
                                    u = co * COHORT + ul     # user index on this core
                                    rt = rpool.tile([H, C], r_dt, name=f"rep{rep}_R_{u}",
                                                    tag="R")
                                    if (ul * ACT_NUM) % ACT_DEN < ACT_NUM:
                                        nc.scalar.activation(
                                            rt[:, :], hxbT_sb[:, :], RELU,
                                            bias=huT_sb[:, u:u + 1], scale=1.0)
                                    else:
                                        nc.vector.tensor_scalar(
                                            rt[:, :], hxbT_sb[:, :],
                                            huT_sb[:, u:u + 1], 0.0, ADD, MAX)
                                    lhsT = w2big_sb[:, 32 * r:32 * r + 32]
                                    for h in range(2):
                                        rhs = rt[:, h * HALF:(h + 1) * HALF]
                                        out = pbank[j][h][32 * j:32 * j + 32, :]
                                        if R_DT == "f32" and MV_F32R:
                                            lhsT_mm = lhsT.bitcast(mybir.dt.float32r)
                                            rhs_mm = rhs.bitcast(mybir.dt.float32r)
                                        else:
                                            lhsT_mm, rhs_mm = lhsT, rhs
                                        nc.tensor.matmul(out, lhsT_mm, rhs_mm,
                                                         start=(r == 0),
                                                         stop=(r == WAVES - 1),
                                                         tile_position=(0, 32 * j))
                            # evict: +b2, PSUM band -> SBUF tile -> one DMA per half
                            for h in range(2):
                                ev = evpool.tile([H, HALF], F32, name=f"rep{rep}_ev_{co}_{h}",
                                                 tag="ev")
                                for j in range(GROUPS):
                                    src = pbank[j][h][32 * j:32 * j + 32, :]
                                    dst = ev[32 * j:32 * j + 32, :]
                                    bslice = b2col_sb[32 * j:32 * j + 32, 0:1]
                                    if EV_ENG == "act":
                                        nc.scalar.activation(
                                            dst, src,
                                            mybir.ActivationFunctionType.Identity,
                                            bias=bslice, scale=1.0)
                                    else:
                                        nc.vector.tensor_scalar(
                                            dst, src, bslice, None, ADD)
                                nc.sync.dma_start(
                                    logits_d[co * COHORT:(co + 1) * COHORT,
                                             h * HALF:(h + 1) * HALF],
                                    ev[:, :])

    nc.compile()
    _CACHE[key] = nc
    return nc


def prep_inputs(h_u_bar, item_content, cold_ids, Wi, bi, W1, b1, W2, b2):
    """Host-side shard/replicate prep. Returns per-core in_maps."""
    f32 = np.float32
    h_u_bar = np.asarray(h_u_bar, f32)
    item_content = np.asarray(item_content, f32)
    cold_ids = np.asarray(cold_ids)
    Wi = np.asarray(Wi, f32)
    bi = np.asarray(bi, f32)
    W1 = np.asarray(W1, f32)
    b1 = np.asarray(b1, f32)
    W2 = np.asarray(W2, f32)
    b2 = np.asarray(b2, f32)

    xcg = item_content[cold_ids]                       # (C, CD) gather
    xcgT = np.ascontiguousarray(xcg.T).astype(np.float16)
    wiT = np.ascontiguousarray(Wi.T).astype(np.float16)
    w1hT = np.ascontiguousarray(W1[:, :D].T)           # (D, H)
    w1xT = np.ascontiguousarray(W1[:, D:].T)           # (D, H)
    b1col = np.ascontiguousarray(b1[:, None])
    bicol = np.ascontiguousarray(bi[:, None])
    b2col = np.full((H, 1), b2[0], f32)

    w2_np = {"f32": np.float32, "f16": np.float16}[R_DT]
    w2big = np.zeros((H, (WAVES + 1) * 32), w2_np)
    for r in range(WAVES):
        w2big[:, 32 * r + r] = W2[0].astype(w2_np)

    common = dict(xcgT=xcgT, wiT=wiT, bicol=bicol, w1xT=w1xT, w1hT=w1hT,
                  b1col=b1col, w2big=w2big, b2col=b2col)
    in_maps = []
    for c in range(N_CORES):
        huT = np.ascontiguousarray(h_u_bar[c * UL:(c + 1) * UL].T)  # (D, UL)
        in_maps.append(dict(common, huT=huT))
    return in_maps


LAST_RESULTS = None  # BassKernelResults of the most recent run (for test.py)


def kernel(h_u_bar, item_content, cold_ids, Wi, bi, W1, b1, W2, b2,
           trace=False, trace_kwargs=None):
    global LAST_RESULTS
    from concourse.bass_utils import run_bass_kernel_spmd

    nc = build_bass()
    in_maps = prep_inputs(h_u_bar, item_content, cold_ids, Wi, bi, W1, b1,
                          W2, b2)
    kw = {}
    if trace:
        kw["trace"] = True
        if trace_kwargs:
            kw.update(trace_kwargs)
    res = run_bass_kernel_spmd(nc, in_maps, core_ids=list(range(N_CORES)), **kw)
    LAST_RESULTS = res
    out = np.concatenate([res.results[c]["logits"] for c in range(N_CORES)],
                         axis=0)
    return out.astype(np.float32)



# revision 20
# speedup vs baseline: 1.2300x; 1.2300x over previous
"""Trainium2 Bass kernel for CGRCNet-style cold-item scoring.

Computes, for U=2048 users and C=1024 cold items:
    x        = item_content @ Wi.T + bi          (only the cold rows are needed)
    xc       = x[cold_ids]                        (C, D)
    hu       = h_u_bar @ W1h.T                    (U, H)
    hx       = xc @ W1x.T                         (C, H)
    logits   = einsum('uch,h->uc', relu(hu[:,None,:] + hx[None,:,:] + b1), W2[0]) + b2

Sharding: U across 8 cores (256 users/core); everything else replicated.
The cold-row gather (zero FLOPs) happens on the host as part of input
distribution; all matrix math runs on device.

Device-side plan (per core), layouts are transposed so H lives on partitions:
  stage 1: xcT  (D=64p,  C=1024f) = WiT.T @ xcgT   (K=300 in 3 chunks) + bi
  stage 2: hxbT (H=128p, C=1024f) = W1xT.T @ xcT + b1
  stage 3: huT  (H=128p, U=256f)  = W1hT.T @ huT_in
  main loop over users u:
     R_u = relu(hxbT + huT[:,u])    one fused op (DVE tensor_scalar add+max,
                                    or ACT activation(Relu, bias=) for a split)
     logits[u, :] = W2 . R_u        PE matvec; the stationary operand is a
                                    (128, 32) slice of "w2big" whose single
                                    nonzero column selects the PSUM partition,
                                    so 32 users accumulate into one col-group
                                    and 128 users pack densely into one bank.
  evict PSUM bank (+b2) -> SBUF -> DMA to HBM.
"""

import os
import numpy as np

# ---------------- problem constants (hardcoded per contract) ----------------
U, D = 2048, 64
I_ITEMS, CD = 50000, 300
C = 1024
H = 128
N_CORES = 8
UL = U // N_CORES            # 256 users per core
COHORT = 128                 # users per PSUM-bank pair
WAVES = 32                   # users per col-group (accumulation depth)
GROUPS = 4                   # PE col groups (32 partitions each)
HALF = 512                   # free-dim half (PSUM bank = 512 fp32)

# ---------------- tunables ----------------
# dtype of the stored hxbT operand ("f32" accurate / "f16" fast DVE 4x mode)
HXB_DT = os.environ.get("KRN_HXB_DT", "f16")
# dtype of the relu output R / matvec operands ("f16" -> 1cyc/row PE; "f32")
R_DT = os.environ.get("KRN_R_DT", "f16")
# when R_DT == "f32": bitcast matvec operands to float32r (1 cyc/row on PE)
MV_F32R = os.environ.get("KRN_MV_F32R", "1") == "1"
# fraction of users whose elementwise op runs on ACT instead of DVE: num/den
ACT_NUM = int(os.environ.get("KRN_ACT_NUM", "7"))
ACT_DEN = 32
# users (of 128 per cohort) whose relu runs on the gpsimd/Pool engine
POOL_NUM = int(os.environ.get("KRN_POOL_NUM", "0"))
# engine for PSUM->SBUF evictions: "act" or "dve"
EV_ENG = os.environ.get("KRN_EV_ENG", "dve")
# dense PSUM packing: interleave all 4 col groups in one bank pair per cohort
# (zero-weight init wave makes has_written semantics robust); f16 path only
DENSE_EV = os.environ.get("KRN_DENSE_EV", "0") == "1"
# stage-2/3 matmuls via float32r bitcast (faster, but exercises f32r codegen)
STG_F32R = os.environ.get("KRN_STG_F32R", "0") == "1"
# bench probes: "" (normal) | "relu_only" (skip matvecs) | "mm_only" (skip relu)
PROBE = os.environ.get("KRN_PROBE", "")
# main-loop structure: "v1" (baseline eviction path) | "v2" (dense bank pair,
# b2 folded into the PSUM init wave, direct PSUM->HBM DMA, no evictions)
MODE = os.environ.get("KRN_MODE", "v1")

_CACHE = {}


def _dt(mybir, s):
    return {"f32": mybir.dt.float32, "f16": mybir.dt.float16}[s]


def build_bass(reps=1, hxb_dts=None, r_dts=None, mv_f32r=None, act_num=None,
               ev_eng=None, dense_ev=None):
    """Build + compile the SPMD single-core program. Returns the Bacc object.

    reps>1 repeats the whole body (benchmarking aid: wall-clock slope vs reps
    isolates device exec time from dispatch overhead)."""
    HXB_DT = hxb_dts if hxb_dts is not None else globals()["HXB_DT"]
    R_DT = r_dts if r_dts is not None else globals()["R_DT"]
    MV_F32R = mv_f32r if mv_f32r is not None else globals()["MV_F32R"]
    ACT_NUM = act_num if act_num is not None else globals()["ACT_NUM"]
    EV_ENG = ev_eng if ev_eng is not None else globals()["EV_ENG"]
    DENSE_EV = dense_ev if dense_ev is not None else globals()["DENSE_EV"]
    if DENSE_EV:
        assert R_DT == "f16", "dense eviction implemented for f16 path only"
    POOL_NUM = globals()["POOL_NUM"]
    key = (HXB_DT, R_DT, MV_F32R, ACT_NUM, EV_ENG, DENSE_EV, STG_F32R, PROBE,
           MODE, POOL_NUM, reps)
    if key in _CACHE:
        return _CACHE[key]

    import concourse.bacc as bacc
    import concourse.mybir as mybir
    from concourse import tile

    F32 = mybir.dt.float32
    hxb_dt = _dt(mybir, HXB_DT)
    r_dt = _dt(mybir, R_DT)
    w2_dt = r_dt
    ADD = mybir.AluOpType.add
    MAX = mybir.AluOpType.max
    RELU = mybir.ActivationFunctionType.Relu

    nc = bacc.Bacc("TRN2", target_bir_lowering=False, debug=False,
                   num_devices=N_CORES)

    # ---- DRAM tensors (names are the in_map keys) ----
    xcgT_d = nc.dram_tensor("xcgT", [CD, C], mybir.dt.float16,
                            kind="ExternalInput").ap()
    wiT_d = nc.dram_tensor("wiT", [CD, D], mybir.dt.float16,
                           kind="ExternalInput").ap()
    bicol_d = nc.dram_tensor("bicol", [D, 1], F32, kind="ExternalInput").ap()
    w1xT_d = nc.dram_tensor("w1xT", [D, H], F32, kind="ExternalInput").ap()
    w1hT_d = nc.dram_tensor("w1hT", [D, H], F32, kind="ExternalInput").ap()
    b1col_d = nc.dram_tensor("b1col", [H, 1], F32, kind="ExternalInput").ap()
    huT_d = nc.dram_tensor("huT", [D, UL], F32, kind="ExternalInput").ap()
    w2big_d = nc.dram_tensor("w2big", [H, (WAVES + 1) * 32], w2_dt,
                             kind="ExternalInput").ap()
    b2col_d = nc.dram_tensor("b2col", [H, 1], F32, kind="ExternalInput").ap()
    if MODE == "v2":
        b2w_d = nc.dram_tensor("b2w", [H, 32], w2_dt, kind="ExternalInput").ap()
        ones_d = nc.dram_tensor("ones", [H, HALF], w2_dt,
                                kind="ExternalInput").ap()
    logits_d = nc.dram_tensor("logits", [UL, C], F32, kind="ExternalOutput").ap()

    KCH = [(0, 128), (128, 128), (256, CD - 256)]  # K chunks of the CD=300 dim

    with tile.TileContext(nc) as tc:
        with (
            tc.tile_pool(name="const", bufs=1) as constp,
            tc.tile_pool(name="work", bufs=1) as workp,
            tc.tile_pool(name="rpool", bufs=8) as rpool,
            tc.tile_pool(name="evpool", bufs=4) as evpool,
        ):
            for rep in range(reps):
                # ---- load replicated operands ----
                xcgT_sb = []
                wiT_sb = []
                for i, (k0, kn) in enumerate(KCH):
                    t = constp.tile([kn, C], mybir.dt.float16, name=f"rep{rep}_xcgT_sb{i}", tag=f"xcg{i}")
                    nc.sync.dma_start(t[:, :], xcgT_d[k0:k0 + kn, :])
                    xcgT_sb.append(t)
                    w = constp.tile([kn, D], mybir.dt.float16, name=f"rep{rep}_wiT_sb{i}", tag=f"wiT{i}")
                    nc.sync.dma_start(w[:, :], wiT_d[k0:k0 + kn, :])
                    wiT_sb.append(w)
                w1xT_sb = constp.tile([D, H], F32, name=f"rep{rep}_w1xT_sb", tag="w1xT")
                nc.sync.dma_start(w1xT_sb[:, :], w1xT_d[:, :])
                w1hT_sb = constp.tile([D, H], F32, name=f"rep{rep}_w1hT_sb", tag="w1hT")
                nc.sync.dma_start(w1hT_sb[:, :], w1hT_d[:, :])
                huTin_sb = constp.tile([D, UL], F32, name=f"rep{rep}_huTin_sb", tag="huTin")
                nc.sync.dma_start(huTin_sb[:, :], huT_d[:, :])
                b1col_sb = constp.tile([H, 1], F32, name=f"rep{rep}_b1col_sb", tag="b1col")
                nc.sync.dma_start(b1col_sb[:, :], b1col_d[:, :])
                bicol_sb = constp.tile([D, 1], F32, name=f"rep{rep}_bicol_sb", tag="bicol")
                nc.sync.dma_start(bicol_sb[:, :], bicol_d[:, :])
                b2col_sb = constp.tile([H, 1], F32, name=f"rep{rep}_b2col_sb", tag="b2col")
                nc.sync.dma_start(b2col_sb[:, :], b2col_d[:, :])
                w2big_sb = constp.tile([H, (WAVES + 1) * 32], w2_dt, name=f"rep{rep}_w2big_sb", tag="w2big")
                nc.sync.dma_start(w2big_sb[:, :], w2big_d[:, :])

                # ---- stages use a small psum pool that is released before the
                # main loop (PSUM budget: 8 banks total) ----
                xcT_sb = workp.tile([D, C], F32, name=f"rep{rep}_xcT_sb", tag="xcT")
                hxbT_sb = workp.tile([H, C], hxb_dt, name=f"rep{rep}_hxbT_sb", tag="hxbT")
                huT_sb = workp.tile([H, UL], F32, name=f"rep{rep}_huT_sb", tag="huT")
                if PROBE == "relu_only":
                    # probe: skip the PE stages; fill operands via DMA so the
                    # relu stream is the only engine work being timed.
                    nc.sync.dma_start(hxbT_sb[:, :], xcgT_d[0:H, :])
                    nc.sync.dma_start(huT_sb[0:D, :], huT_d[:, :])
                    nc.sync.dma_start(huT_sb[D:H, :], huT_d[:, :])
                elif True:
                  with tc.tile_pool(name=f"rep{rep}_pstg", bufs=2, space="PSUM") as pstg:
                    # stage 1: xcT (64, 1024) = WiT.T @ xcgT + bi
                    for n in range(2):
                        ps1 = pstg.tile([H, HALF], F32, name=f"rep{rep}_ps_s1_{n}",
                                        tag="pstg")
                        for k, (k0, kn) in enumerate(KCH):
                            nc.tensor.matmul(
                                ps1[0:D, :], wiT_sb[k][:, :],
                                xcgT_sb[k][:, n * HALF:(n + 1) * HALF],
                                start=(k == 0), stop=(k == len(KCH) - 1),
                            )
                        if EV_ENG == "act":
                            nc.scalar.activation(
                                xcT_sb[:, n * HALF:(n + 1) * HALF], ps1[0:D, :],
                                mybir.ActivationFunctionType.Identity,
                                bias=bicol_sb[:, 0:1], scale=1.0)
                        else:
                            nc.vector.tensor_scalar(
                                xcT_sb[:, n * HALF:(n + 1) * HALF], ps1[0:D, :],
                                bicol_sb[:, 0:1], None, ADD)

                    # stage 2: hxbT (128, 1024) = W1xT.T @ xcT + b1
                    for n in range(2):
                        ps2 = pstg.tile([H, HALF], F32, name=f"rep{rep}_ps_s2_{n}",
                                        tag="pstg")
                        if STG_F32R:
                            nc.tensor.matmul(
                                ps2[:, :],
                                w1xT_sb[:, :].bitcast(mybir.dt.float32r),
                                xcT_sb[:, n * HALF:(n + 1) * HALF].bitcast(
                                    mybir.dt.float32r),
                                start=True, stop=True)
                        else:
                            nc.tensor.matmul(
                                ps2[:, :], w1xT_sb[:, :],
                                xcT_sb[:, n * HALF:(n + 1) * HALF],
                                start=True, stop=True)
                        if EV_ENG == "act":
                            nc.scalar.activation(
                                hxbT_sb[:, n * HALF:(n + 1) * HALF], ps2[:, :],
                                mybir.ActivationFunctionType.Identity,
                                bias=b1col_sb[:, 0:1], scale=1.0)
                        else:
                            nc.vector.tensor_scalar(
                                hxbT_sb[:, n * HALF:(n + 1) * HALF], ps2[:, :],
                                b1col_sb[:, 0:1], None, ADD)

                    # stage 3: huT (128, 256) = W1hT.T @ huT_in
                    ps3 = pstg.tile([H, HALF], F32, name=f"rep{rep}_ps_s3", tag="pstg")
                    if STG_F32R:
                        nc.tensor.matmul(
                            ps3[:, 0:UL],
                            w1hT_sb[:, :].bitcast(mybir.dt.float32r),
                            huTin_sb[:, :].bitcast(mybir.dt.float32r),
                            start=True, stop=True)
                    else:
                        nc.tensor.matmul(
                            ps3[:, 0:UL], w1hT_sb[:, :], huTin_sb[:, :],
                            start=True, stop=True)
                    if EV_ENG == "act":
                        nc.scalar.copy(huT_sb[:, :], ps3[:, 0:UL])
                    else:
                        nc.vector.tensor_copy(huT_sb[:, :], ps3[:, 0:UL])

                # ---- main loop ----
                # Each (col-group j, half h) owns a full PSUM bank; group j's 32
                # users accumulate into partitions [32j, 32j+32) of its bank via
                # the shifted-column stationary operand. One accumulation group
                # per bank -> well-defined has_written semantics.
                n_cohorts = UL // COHORT
                if MODE == "v2":
                    # per-cohort-slot engine assignment for the relu stream
                    emap = ["d"] * COHORT
                    if POOL_NUM:
                        st = COHORT / POOL_NUM
                        for i in range(POOL_NUM):
                            emap[int(i * st)] = "p"
                    rest = [x for x in range(COHORT) if emap[x] == "d"]
                    na = round(COHORT * ACT_NUM / ACT_DEN)
                    if na:
                        st = len(rest) / na
                        for i in range(na):
                            emap[rest[int(i * st)]] = "a"
                    b2w_sb = constp.tile([H, 32], w2_dt,
                                         name=f"rep{rep}_b2w_sb", tag="b2w")
                    nc.sync.dma_start(b2w_sb[:, :], b2w_d[:, :])
                    ones_sb = constp.tile([H, HALF], w2_dt,
                                          name=f"rep{rep}_ones_sb", tag="ones")
                    nc.sync.dma_start(ones_sb[:, :], ones_d[:, :])
                    with tc.tile_pool(name=f"rep{rep}_plogv", bufs=8,
                                      space="PSUM") as plog:
                        for co in range(n_cohorts):
                            pbank = [plog.tile([H, HALF], F32,
                                               name=f"rep{rep}_plv_{co}_{h}",
                                               tag="plv") for h in range(2)]
                            # init wave: PSUM := b2 everywhere (b2/H weights
                            # against a ones stream), one group per quadrant
                            for j in range(GROUPS):
                                for h in range(2):
                                    nc.tensor.matmul(
                                        pbank[h][32 * j:32 * j + 32, :],
                                        b2w_sb[:, :], ones_sb[:, :],
                                        start=True, stop=False,
                                        tile_position=(0, 32 * j),
                                        skip_group_check=True)
                            for r in range(WAVES):
                                for j in range(GROUPS):
                                    ul = 32 * j + r
                                    u = co * COHORT + ul
                                    rt = rpool.tile([H, C], r_dt,
                                                    name=f"rep{rep}_R_{u}",
                                                    tag="R")
                                    eng = emap[ul]
                                    if eng == "a":
                                        nc.scalar.activation(
                                            rt[:, :], hxbT_sb[:, :], RELU,
                                            bias=huT_sb[:, u:u + 1], scale=1.0)
                                    elif eng == "p":
                                        nc.gpsimd.tensor_scalar(
                                            rt[:, :], hxbT_sb[:, :],
                                            huT_sb[:, u:u + 1], 0.0, ADD, MAX)
                                    else:
                                        nc.vector.tensor_scalar(
                                            rt[:, :], hxbT_sb[:, :],
                                            huT_sb[:, u:u + 1], 0.0, ADD, MAX)
                                    lhsT = w2big_sb[:, 32 * r:32 * r + 32]
                                    for h in range(2):
                                        nc.tensor.matmul(
                                            pbank[h][32 * j:32 * j + 32, :],
                                            lhsT,
                                            rt[:, h * HALF:(h + 1) * HALF],
                                            start=False,
                                            stop=(r == WAVES - 1),
                                            tile_position=(0, 32 * j),
                                            skip_group_check=True)
                            for h in range(2):
                                ev = evpool.tile([H, HALF], F32,
                                                 name=f"rep{rep}_evv_{co}_{h}",
                                                 tag="ev")
                                if h == 0:
                                    nc.vector.tensor_copy(ev[:, :],
                                                          pbank[h][:, :])
                                else:
                                    nc.scalar.copy(ev[:, :], pbank[h][:, :])
                                nc.sync.dma_start(
                                    logits_d[co * COHORT:(co + 1) * COHORT,
                                             h * HALF:(h + 1) * HALF],
                                    ev[:, :])
                elif DENSE_EV:
                    with tc.tile_pool(name=f"rep{rep}_plogd", bufs=4,
                                      space="PSUM") as plog:
                        zsl = w2big_sb[:, WAVES * 32:WAVES * 32 + 32]
                        for co in range(n_cohorts):
                            pbank = [plog.tile([H, HALF], F32,
                                               name=f"rep{rep}_plogd_{co}_{h}",
                                               tag="plogd") for h in range(2)]
                            for j in range(GROUPS):
                                for h in range(2):
                                    nc.tensor.matmul(
                                        pbank[h][32 * j:32 * j + 32, :], zsl,
                                        hxbT_sb[:, h * HALF:(h + 1) * HALF],
                                        start=True, stop=False,
                                        tile_position=(0, 32 * j),
                                        skip_group_check=True)
                            for r in range(WAVES):
                                for j in range(GROUPS):
                                    ul = 32 * j + r
                                    u = co * COHORT + ul
                                    rt = rpool.tile([H, C], r_dt,
                                                    name=f"rep{rep}_R_{u}",
                                                    tag="R")
                                    if (ul * ACT_NUM) % ACT_DEN < ACT_NUM:
                                        nc.scalar.activation(
                                            rt[:, :], hxbT_sb[:, :], RELU,
                                            bias=huT_sb[:, u:u + 1], scale=1.0)
                                    else:
                                        nc.vector.tensor_scalar(
                                            rt[:, :], hxbT_sb[:, :],
                                            huT_sb[:, u:u + 1], 0.0, ADD, MAX)
                                    lhsT = w2big_sb[:, 32 * r:32 * r + 32]
                                    for h in range(2):
                                        nc.tensor.matmul(
                                            pbank[h][32 * j:32 * j + 32, :],
                                            lhsT,
                                            rt[:, h * HALF:(h + 1) * HALF],
                                            start=False,
                                            stop=(r == WAVES - 1),
                                            tile_position=(0, 32 * j),
                                            skip_group_check=True)
                            for h in range(2):
                                ev = evpool.tile([H, HALF], F32,
                                                 name=f"rep{rep}_evd_{co}_{h}",
                                                 tag="ev")
                                if EV_ENG == "act":
                                    nc.scalar.activation(
                                        ev[:, :], pbank[h][:, :],
                                        mybir.ActivationFunctionType.Identity,
                                        bias=b2col_sb[:, 0:1], scale=1.0)
                                else:
                                    nc.vector.tensor_scalar(
                                        ev[:, :], pbank[h][:, :],
                                        b2col_sb[:, 0:1], None, ADD)
                                nc.sync.dma_start(
                                    logits_d[co * COHORT:(co + 1) * COHORT,
                                             h * HALF:(h + 1) * HALF],
                                    ev[:, :])
                else:
                    rtst = None
                    if PROBE == "mm_only":
                        rtst = []
                        for i in range(8):
                            t = workp.tile([H, C], r_dt,
                                           name=f"rep{rep}_rtst{i}",
                                           tag=f"rtst{i}")
                            nc.sync.dma_start(t[:, :], xcgT_d[0:H, :])
                            rtst.append(t)
                    with tc.tile_pool(name=f"rep{rep}_plog", bufs=8, space="PSUM") as plog:
                        for co in range(n_cohorts):
                            pbank = None
                            if PROBE != "relu_only":
                                pbank = [[plog.tile([H, HALF], F32,
                                                    name=f"rep{rep}_plog_{co}_{j}_{h}", tag="plog")
                                          for h in range(2)] for j in range(GROUPS)]
                            for r in range(WAVES):
                                for j in range(GROUPS):
                                    ul = 32 * j + r          # partition within cohort
                                    u = co * COHORT + ul     # user index on this core
                                    if PROBE == "mm_only":
                                        rt = rtst[(r * GROUPS + j) % 8]
                                    else:
                                        rt = rpool.tile([H, C], r_dt,
                                                        name=f"rep{rep}_R_{u}",
                                                        tag="R")
                                    if PROBE != "mm_only":
                                        if (ul * ACT_NUM) % ACT_DEN < ACT_NUM:
                                            nc.scalar.activation(
                                                rt[:, :], hxbT_sb[:, :], RELU,
                                                bias=huT_sb[:, u:u + 1], scale=1.0)
                                        else:
                                            nc.vector.tensor_scalar(
                                                rt[:, :], hxbT_sb[:, :],
                                                huT_sb[:, u:u + 1], 0.0, ADD, MAX)
                                    lhsT = w2big_sb[:, 32 * r:32 * r + 32]
                                    for h in range(2):
                                        if PROBE == "relu_only":
                                            continue
                                        rhs = rt[:, h * HALF:(h + 1) * HALF]
                                        out = pbank[j][h][32 * j:32 * j + 32, :]
                                        if R_DT == "f32" and MV_F32R:
                                            lhsT_mm = lhsT.bitcast(mybir.dt.float32r)
                                            rhs_mm = rhs.bitcast(mybir.dt.float32r)
                                        else:
                                            lhsT_mm, rhs_mm = lhsT, rhs
                                        nc.tensor.matmul(out, lhsT_mm, rhs_mm,
                                                         start=(r == 0),
                                                         stop=(r == WAVES - 1),
                                                         tile_position=(0, 32 * j))
                            # evict: +b2, PSUM band -> SBUF tile -> one DMA per half
                            for h in range(2):
                                if PROBE == "relu_only":
                                    continue
                                ev = evpool.tile([H, HALF], F32, name=f"rep{rep}_ev_{co}_{h}",
                                                 tag="ev")
                                for j in range(GROUPS):
                                    src = pbank[j][h][32 * j:32 * j + 32, :]
                                    dst = ev[32 * j:32 * j + 32, :]
                                    bslice = b2col_sb[32 * j:32 * j + 32, 0:1]
                                    if EV_ENG == "act":
                                        nc.scalar.activation(
                                            dst, src,
                                            mybir.ActivationFunctionType.Identity,
                                            bias=bslice, scale=1.0)
                                    else:
                                        nc.vector.tensor_scalar(
                                            dst, src, bslice, None, ADD)
                                nc.sync.dma_start(
                                    logits_d[co * COHORT:(co + 1) * COHORT,
                                             h * HALF:(h + 1) * HALF],
                                    ev[:, :])

    nc.compile()
    _CACHE[key] = nc
    return nc


def prep_inputs(h_u_bar, item_content, cold_ids, Wi, bi, W1, b1, W2, b2):
    """Host-side shard/replicate prep. Returns per-core in_maps."""
    f32 = np.float32
    h_u_bar = np.asarray(h_u_bar, f32)
    item_content = np.asarray(item_content, f32)
    cold_ids = np.asarray(cold_ids)
    Wi = np.asarray(Wi, f32)
    bi = np.asarray(bi, f32)
    W1 = np.asarray(W1, f32)
    b1 = np.asarray(b1, f32)
    W2 = np.asarray(W2, f32)
    b2 = np.asarray(b2, f32)

    xcg = item_content[cold_ids]                       # (C, CD) gather
    xcgT = np.ascontiguousarray(xcg.T).astype(np.float16)
    wiT = np.ascontiguousarray(Wi.T).astype(np.float16)
    w1hT = np.ascontiguousarray(W1[:, :D].T)           # (D, H)
    w1xT = np.ascontiguousarray(W1[:, D:].T)           # (D, H)
    b1col = np.ascontiguousarray(b1[:, None])
    bicol = np.ascontiguousarray(bi[:, None])
    b2col = np.full((H, 1), b2[0], f32)

    w2_np = {"f32": np.float32, "f16": np.float16}[R_DT]
    w2big = np.zeros((H, (WAVES + 1) * 32), w2_np)
    for r in range(WAVES):
        w2big[:, 32 * r + r] = W2[0].astype(w2_np)

    common = dict(xcgT=xcgT, wiT=wiT, bicol=bicol, w1xT=w1xT, w1hT=w1hT,
                  b1col=b1col, w2big=w2big, b2col=b2col)
    if MODE == "v2":
        common["b2w"] = np.full((H, 32), b2[0] / H, w2_np)
        common["ones"] = np.ones((H, HALF), w2_np)
    in_maps = []
    for c in range(N_CORES):
        huT = np.ascontiguousarray(h_u_bar[c * UL:(c + 1) * UL].T)  # (D, UL)
        in_maps.append(dict(common, huT=huT))
    return in_maps


LAST_RESULTS = None  # BassKernelResults of the most recent run (for test.py)


def kernel(h_u_bar, item_content, cold_ids, Wi, bi, W1, b1, W2, b2,
           trace=False, trace_kwargs=None):
    global LAST_RESULTS
    from concourse.bass_utils import run_bass_kernel_spmd

    nc = build_bass()
    in_maps = prep_inputs(h_u_bar, item_content, cold_ids, Wi, bi, W1, b1,
                          W2, b2)
    kw = {}
    if trace:
        kw["trace"] = True
        if trace_kwargs:
            kw.update(trace_kwargs)
    res = run_bass_kernel_spmd(nc, in_maps, core_ids=list(range(N_CORES)), **kw)
    LAST_RESULTS = res
    out = np.concatenate([res.results[c]["logits"] for c in range(N_CORES)],
                         axis=0)
    return out.astype(np.float32)



# revision 34
# speedup vs baseline: 1.2505x; 1.0167x over previous
"""Trainium2 Bass kernel for CGRCNet-style cold-item scoring.

Computes, for U=2048 users and C=1024 cold items:
    x        = item_content @ Wi.T + bi          (only the cold rows are needed)
    xc       = x[cold_ids]                        (C, D)
    hu       = h_u_bar @ W1h.T                    (U, H)
    hx       = xc @ W1x.T                         (C, H)
    logits   = einsum('uch,h->uc', relu(hu[:,None,:] + hx[None,:,:] + b1), W2[0]) + b2

Sharding: U across 8 cores (256 users/core); everything else replicated.
The cold-row gather (zero FLOPs) happens on the host as part of input
distribution; all matrix math runs on device.

Device-side plan (per core), layouts are transposed so H lives on partitions:
  stage 1: xcT  (D=64p,  C=1024f) = WiT.T @ xcgT   (K=300 in 3 chunks) + bi
  stage 2: hxbT (H=128p, C=1024f) = W1xT.T @ xcT + b1
  stage 3: huT  (H=128p, U=256f)  = W1hT.T @ huT_in
  main loop over users u:
     R_u = relu(hxbT + huT[:,u])    one fused op (DVE tensor_scalar add+max,
                                    or ACT activation(Relu, bias=) for a split)
     logits[u, :] = W2 . R_u        PE matvec; the stationary operand is a
                                    (128, 32) slice of "w2big" whose single
                                    nonzero column selects the PSUM partition,
                                    so 32 users accumulate into one col-group
                                    and 128 users pack densely into one bank.
  evict PSUM bank (+b2) -> SBUF -> DMA to HBM.
"""

import os
import numpy as np

# ---------------- problem constants (hardcoded per contract) ----------------
U, D = 2048, 64
I_ITEMS, CD = 50000, 300
C = 1024
H = 128
N_CORES = 8
UL = U // N_CORES            # 256 users per core
COHORT = 128                 # users per PSUM-bank pair
WAVES = 32                   # users per col-group (accumulation depth)
GROUPS = 4                   # PE col groups (32 partitions each)
HALF = 512                   # free-dim half (PSUM bank = 512 fp32)

# ---------------- tunables ----------------
# dtype of the stored hxbT operand ("f32" accurate / "f16" fast DVE 4x mode)
HXB_DT = os.environ.get("KRN_HXB_DT", "f16")
# dtype of the relu output R / matvec operands ("f16" -> 1cyc/row PE; "f32")
R_DT = os.environ.get("KRN_R_DT", "f16")
# when R_DT == "f32": bitcast matvec operands to float32r (1 cyc/row on PE)
MV_F32R = os.environ.get("KRN_MV_F32R", "1") == "1"
# fraction of users whose elementwise op runs on ACT instead of DVE: num/den
ACT_NUM = int(os.environ.get("KRN_ACT_NUM", "26"))
ACT_DEN = 32
# users (of 128 per cohort) whose relu runs on the gpsimd/Pool engine
POOL_NUM = int(os.environ.get("KRN_POOL_NUM", "0"))
# engine for PSUM->SBUF evictions: "act" or "dve"
EV_ENG = os.environ.get("KRN_EV_ENG", "dve")
# dense PSUM packing: interleave all 4 col groups in one bank pair per cohort
# (zero-weight init wave makes has_written semantics robust); f16 path only
DENSE_EV = os.environ.get("KRN_DENSE_EV", "0") == "1"
# stage-2/3 matmuls via float32r bitcast (faster, but exercises f32r codegen)
STG_F32R = os.environ.get("KRN_STG_F32R", "0") == "1"
# v2: tiled stage head (col-tiled stage1 halves, row-tiled stage2 halves)
STG_TILE = os.environ.get("KRN_STG_TILE", "1") == "1"
# relu tile pool depth (cross-engine lookahead)
RBUFS = int(os.environ.get("KRN_RBUFS", "12"))
# bench probes: "" (normal) | "relu_only" (skip matvecs) | "mm_only" (skip relu)
PROBE = os.environ.get("KRN_PROBE", "")
# main-loop structure: "v1" (baseline eviction path) | "v2" (dense bank pair,
# b2 folded into the PSUM init wave, direct PSUM->HBM DMA, no evictions)
MODE = os.environ.get("KRN_MODE", "v2")

_CACHE = {}


def _dt(mybir, s):
    return {"f32": mybir.dt.float32, "f16": mybir.dt.float16}[s]


def build_bass(reps=1, hxb_dts=None, r_dts=None, mv_f32r=None, act_num=None,
               ev_eng=None, dense_ev=None):
    """Build + compile the SPMD single-core program. Returns the Bacc object.

    reps>1 repeats the whole body (benchmarking aid: wall-clock slope vs reps
    isolates device exec time from dispatch overhead)."""
    HXB_DT = hxb_dts if hxb_dts is not None else globals()["HXB_DT"]
    R_DT = r_dts if r_dts is not None else globals()["R_DT"]
    MV_F32R = mv_f32r if mv_f32r is not None else globals()["MV_F32R"]
    ACT_NUM = act_num if act_num is not None else globals()["ACT_NUM"]
    EV_ENG = ev_eng if ev_eng is not None else globals()["EV_ENG"]
    DENSE_EV = dense_ev if dense_ev is not None else globals()["DENSE_EV"]
    if DENSE_EV:
        assert R_DT == "f16", "dense eviction implemented for f16 path only"
    POOL_NUM = globals()["POOL_NUM"]
    key = (HXB_DT, R_DT, MV_F32R, ACT_NUM, EV_ENG, DENSE_EV, STG_F32R, PROBE,
           MODE, POOL_NUM, STG_TILE, globals()["RBUFS"], reps)
    if key in _CACHE:
        return _CACHE[key]

    import concourse.bacc as bacc
    import concourse.mybir as mybir
    from concourse import tile

    F32 = mybir.dt.float32
    hxb_dt = _dt(mybir, HXB_DT)
    r_dt = _dt(mybir, R_DT)
    w2_dt = r_dt
    ADD = mybir.AluOpType.add
    MAX = mybir.AluOpType.max
    RELU = mybir.ActivationFunctionType.Relu

    nc = bacc.Bacc("TRN2", target_bir_lowering=False, debug=False,
                   num_devices=N_CORES)

    # ---- DRAM tensors (names are the in_map keys) ----
    stg_dt = mybir.dt.float16 if MODE == "v2" else F32
    xcgT_d = nc.dram_tensor("xcgT", [CD, C], mybir.dt.float16,
                            kind="ExternalInput").ap()
    wiT_d = nc.dram_tensor("wiT", [CD, D], mybir.dt.float16,
                           kind="ExternalInput").ap()
    bicol_d = nc.dram_tensor("bicol", [D, 1], F32, kind="ExternalInput").ap()
    w1xT_d = nc.dram_tensor("w1xT", [D, H], stg_dt, kind="ExternalInput").ap()
    w1hT_d = nc.dram_tensor("w1hT", [D, H], stg_dt, kind="ExternalInput").ap()
    b1col_d = nc.dram_tensor("b1col", [H, 1], F32, kind="ExternalInput").ap()
    huT_d = nc.dram_tensor("huT", [D, UL], stg_dt, kind="ExternalInput").ap()
    w2big_d = nc.dram_tensor("w2big", [H, (WAVES + 1) * 32], w2_dt,
                             kind="ExternalInput").ap()
    b2col_d = nc.dram_tensor("b2col", [H, 1], F32, kind="ExternalInput").ap()
    if MODE == "v2":
        b2w_d = nc.dram_tensor("b2w", [H, 32], w2_dt, kind="ExternalInput").ap()
        ones_d = nc.dram_tensor("ones", [H, HALF], w2_dt,
                                kind="ExternalInput").ap()
        bicol2_d = nc.dram_tensor("bicol2", [H, 1], F32,
                                  kind="ExternalInput").ap()
        w1xT2_d = nc.dram_tensor("w1xT2", [H, H], stg_dt,
                                 kind="ExternalInput").ap()
    logits_d = nc.dram_tensor("logits", [UL, C], F32, kind="ExternalOutput").ap()

    KCH = [(0, 128), (128, 128), (256, CD - 256)]  # K chunks of the CD=300 dim

    with tile.TileContext(nc) as tc:
        with (
            tc.tile_pool(name="const", bufs=1) as constp,
            tc.tile_pool(name="work", bufs=1) as workp,
            tc.tile_pool(name="rpool", bufs=RBUFS) as rpool,
            tc.tile_pool(name="evpool", bufs=4) as evpool,
        ):
            for rep in range(reps):
                # ---- load replicated operands ----
                xcgT_sb = []
                wiT_sb = []
                for i, (k0, kn) in enumerate(KCH):
                    t = constp.tile([kn, C], mybir.dt.float16, name=f"rep{rep}_xcgT_sb{i}", tag=f"xcg{i}")
                    nc.sync.dma_start(t[:, :], xcgT_d[k0:k0 + kn, :])
                    xcgT_sb.append(t)
                    w = constp.tile([kn, D], mybir.dt.float16, name=f"rep{rep}_wiT_sb{i}", tag=f"wiT{i}")
                    nc.sync.dma_start(w[:, :], wiT_d[k0:k0 + kn, :])
                    wiT_sb.append(w)
                w1xT_sb = constp.tile([D, H], stg_dt, name=f"rep{rep}_w1xT_sb", tag="w1xT")
                nc.sync.dma_start(w1xT_sb[:, :], w1xT_d[:, :])
                w1hT_sb = constp.tile([D, H], stg_dt, name=f"rep{rep}_w1hT_sb", tag="w1hT")
                nc.sync.dma_start(w1hT_sb[:, :], w1hT_d[:, :])
                huTin_sb = constp.tile([D, UL], stg_dt, name=f"rep{rep}_huTin_sb", tag="huTin")
                nc.sync.dma_start(huTin_sb[:, :], huT_d[:, :])
                b1col_sb = constp.tile([H, 1], F32, name=f"rep{rep}_b1col_sb", tag="b1col")
                nc.sync.dma_start(b1col_sb[:, :], b1col_d[:, :])
                bicol_sb = constp.tile([D, 1], F32, name=f"rep{rep}_bicol_sb", tag="bicol")
                nc.sync.dma_start(bicol_sb[:, :], bicol_d[:, :])
                b2col_sb = constp.tile([H, 1], F32, name=f"rep{rep}_b2col_sb", tag="b2col")
                nc.sync.dma_start(b2col_sb[:, :], b2col_d[:, :])
                w2big_sb = constp.tile([H, (WAVES + 1) * 32], w2_dt, name=f"rep{rep}_w2big_sb", tag="w2big")
                nc.sync.dma_start(w2big_sb[:, :], w2big_d[:, :])

                # ---- stages use a small psum pool that is released before the
                # main loop (PSUM budget: 8 banks total) ----
                xcT_sb = workp.tile([D, C], stg_dt, name=f"rep{rep}_xcT_sb", tag="xcT")
                hxbT_sb = workp.tile([H, C], hxb_dt, name=f"rep{rep}_hxbT_sb", tag="hxbT")
                huT_sb = workp.tile([H, UL], F32, name=f"rep{rep}_huT_sb", tag="huT")
                if PROBE == "relu_only":
                    # probe: skip the PE stages; fill operands via DMA so the
                    # relu stream is the only engine work being timed.
                    nc.sync.dma_start(hxbT_sb[:, :], xcgT_d[0:H, :])
                    nc.sync.dma_start(huT_sb[0:D, :], huT_d[:, :])
                    nc.sync.dma_start(huT_sb[D:H, :], huT_d[:, :])
                elif MODE == "v2" and STG_TILE:
                    bicol2_sb = constp.tile([H, 1], F32,
                                            name=f"rep{rep}_bicol2_sb",
                                            tag="bicol2")
                    nc.sync.dma_start(bicol2_sb[:, :], bicol2_d[:, :])
                    w1xT2_sb = constp.tile([H, H], stg_dt,
                                           name=f"rep{rep}_w1xT2_sb",
                                           tag="w1xT2")
                    nc.sync.dma_start(w1xT2_sb[:, :], w1xT2_d[:, :])
                    xcT2_sb = workp.tile([H, HALF], stg_dt,
                                         name=f"rep{rep}_xcT2_sb", tag="xcT2")
                    with tc.tile_pool(name=f"rep{rep}_pstg", bufs=3,
                                      space="PSUM") as pstg:
                        # stage 1: the two C-halves run concurrently on col
                        # tiles (0,0)/(0,64); half n lands on partitions 64n+
                        ps1 = pstg.tile([H, HALF], F32,
                                        name=f"rep{rep}_ps_s1", tag="pstg")
                        for n in range(2):
                            for k, (k0, kn) in enumerate(KCH):
                                nc.tensor.matmul(
                                    ps1[64 * n:64 * n + D, :], wiT_sb[k][:, :],
                                    xcgT_sb[k][:, n * HALF:(n + 1) * HALF],
                                    start=(k == 0), stop=(k == len(KCH) - 1),
                                    tile_position=(0, 64 * n),
                                    skip_group_check=True)
                        # stage 3 before stage 2: PE fills the eviction gap
                        ps3 = pstg.tile([H, HALF], F32,
                                        name=f"rep{rep}_ps_s3", tag="pstg")
                        nc.tensor.matmul(ps3[:, 0:UL], w1hT_sb[:, :],
                                         huTin_sb[:, :], start=True, stop=True)
                        nc.vector.tensor_scalar(
                            xcT2_sb[0:D, :], ps1[0:D, :],
                            bicol2_sb[0:D, 0:1], None, ADD)
                        nc.scalar.activation(
                            xcT2_sb[D:H, :], ps1[D:H, :],
                            mybir.ActivationFunctionType.Identity,
                            bias=bicol2_sb[D:H, 0:1], scale=1.0)
                        nc.vector.tensor_copy(huT_sb[:, :], ps3[:, 0:UL])
                        # stage 2: the two C-halves on row tiles (0,0)/(64,0)
                        for n in range(2):
                            ps2 = pstg.tile([H, HALF], F32,
                                            name=f"rep{rep}_ps_s2_{n}",
                                            tag="pstg")
                            nc.tensor.matmul(
                                ps2[:, :], w1xT2_sb[64 * n:64 * n + D, :],
                                xcT2_sb[64 * n:64 * n + D, :],
                                start=True, stop=True,
                                tile_position=(64 * n, 0),
                                skip_group_check=True)
                            if n == 0:
                                nc.vector.tensor_scalar(
                                    hxbT_sb[:, n * HALF:(n + 1) * HALF],
                                    ps2[:, :], b1col_sb[:, 0:1], None, ADD)
                            else:
                                nc.scalar.activation(
                                    hxbT_sb[:, n * HALF:(n + 1) * HALF],
                                    ps2[:, :],
                                    mybir.ActivationFunctionType.Identity,
                                    bias=b1col_sb[:, 0:1], scale=1.0)
                elif True:
                  with tc.tile_pool(name=f"rep{rep}_pstg", bufs=2, space="PSUM") as pstg:
                    # stage 1: xcT (64, 1024) = WiT.T @ xcgT + bi
                    for n in range(2):
                        ps1 = pstg.tile([H, HALF], F32, name=f"rep{rep}_ps_s1_{n}",
                                        tag="pstg")
                        for k, (k0, kn) in enumerate(KCH):
                            nc.tensor.matmul(
                                ps1[0:D, :], wiT_sb[k][:, :],
                                xcgT_sb[k][:, n * HALF:(n + 1) * HALF],
                                start=(k == 0), stop=(k == len(KCH) - 1),
                            )
                        if EV_ENG == "act":
                            nc.scalar.activation(
                                xcT_sb[:, n * HALF:(n + 1) * HALF], ps1[0:D, :],
                                mybir.ActivationFunctionType.Identity,
                                bias=bicol_sb[:, 0:1], scale=1.0)
                        else:
                            nc.vector.tensor_scalar(
                                xcT_sb[:, n * HALF:(n + 1) * HALF], ps1[0:D, :],
                                bicol_sb[:, 0:1], None, ADD)

                    # stage 2: hxbT (128, 1024) = W1xT.T @ xcT + b1
                    for n in range(2):
                        ps2 = pstg.tile([H, HALF], F32, name=f"rep{rep}_ps_s2_{n}",
                                        tag="pstg")
                        if STG_F32R:
                            nc.tensor.matmul(
                                ps2[:, :],
                                w1xT_sb[:, :].bitcast(mybir.dt.float32r),
                                xcT_sb[:, n * HALF:(n + 1) * HALF].bitcast(
                                    mybir.dt.float32r),
                                start=True, stop=True)
                        else:
                            nc.tensor.matmul(
                                ps2[:, :], w1xT_sb[:, :],
                                xcT_sb[:, n * HALF:(n + 1) * HALF],
                                start=True, stop=True)
                        if EV_ENG == "act":
                            nc.scalar.activation(
                                hxbT_sb[:, n * HALF:(n + 1) * HALF], ps2[:, :],
                                mybir.ActivationFunctionType.Identity,
                                bias=b1col_sb[:, 0:1], scale=1.0)
                        else:
                            nc.vector.tensor_scalar(
                                hxbT_sb[:, n * HALF:(n + 1) * HALF], ps2[:, :],
                                b1col_sb[:, 0:1], None, ADD)

                    # stage 3: huT (128, 256) = W1hT.T @ huT_in
                    ps3 = pstg.tile([H, HALF], F32, name=f"rep{rep}_ps_s3", tag="pstg")
                    if STG_F32R:
                        nc.tensor.matmul(
                            ps3[:, 0:UL],
                            w1hT_sb[:, :].bitcast(mybir.dt.float32r),
                            huTin_sb[:, :].bitcast(mybir.dt.float32r),
                            start=True, stop=True)
                    else:
                        nc.tensor.matmul(
                            ps3[:, 0:UL], w1hT_sb[:, :], huTin_sb[:, :],
                            start=True, stop=True)
                    if EV_ENG == "act":
                        nc.scalar.copy(huT_sb[:, :], ps3[:, 0:UL])
                    else:
                        nc.vector.tensor_copy(huT_sb[:, :], ps3[:, 0:UL])

                # ---- main loop ----
                # Each (col-group j, half h) owns a full PSUM bank; group j's 32
                # users accumulate into partitions [32j, 32j+32) of its bank via
                # the shifted-column stationary operand. One accumulation group
                # per bank -> well-defined has_written semantics.
                n_cohorts = UL // COHORT
                if MODE == "v2":
                    # per-cohort-slot engine assignment for the relu stream
                    emap = ["d"] * COHORT
                    if POOL_NUM:
                        st = COHORT / POOL_NUM
                        for i in range(POOL_NUM):
                            emap[int(i * st)] = "p"
                    rest = [x for x in range(COHORT) if emap[x] == "d"]
                    # ACT_NUM < 16: legacy num/32 fraction; >= 16: direct
                    # per-cohort user count (finer resolution)
                    na = (ACT_NUM if ACT_NUM >= 16
                          else round(COHORT * ACT_NUM / ACT_DEN))
                    if na:
                        st = len(rest) / na
                        for i in range(na):
                            emap[rest[int(i * st)]] = "a"
                    b2w_sb = constp.tile([H, 32], w2_dt,
                                         name=f"rep{rep}_b2w_sb", tag="b2w")
                    nc.sync.dma_start(b2w_sb[:, :], b2w_d[:, :])
                    ones_sb = constp.tile([H, HALF], w2_dt,
                                          name=f"rep{rep}_ones_sb", tag="ones")
                    nc.sync.dma_start(ones_sb[:, :], ones_d[:, :])
                    with tc.tile_pool(name=f"rep{rep}_plogv", bufs=8,
                                      space="PSUM") as plog:
                        for co in range(n_cohorts):
                            pbank = [plog.tile([H, HALF], F32,
                                               name=f"rep{rep}_plv_{co}_{h}",
                                               tag="plv") for h in range(2)]
                            # init wave: PSUM := b2 everywhere (b2/H weights
                            # against a ones stream), one group per quadrant
                            for j in range(GROUPS):
                                for h in range(2):
                                    nc.tensor.matmul(
                                        pbank[h][32 * j:32 * j + 32, :],
                                        b2w_sb[:, :], ones_sb[:, :],
                                        start=True, stop=False,
                                        tile_position=(0, 32 * j),
                                        skip_group_check=True)
                            for r in range(WAVES):
                                for j in range(GROUPS):
                                    ul = 32 * j + r
                                    u = co * COHORT + ul
                                    rt = rpool.tile([H, C], r_dt,
                                                    name=f"rep{rep}_R_{u}",
                                                    tag="R")
                                    eng = emap[ul]
                                    if eng == "a":
                                        nc.scalar.activation(
                                            rt[:, :], hxbT_sb[:, :], RELU,
                                            bias=huT_sb[:, u:u + 1], scale=1.0)
                                    elif eng == "p":
                                        nc.gpsimd.tensor_scalar(
                                            rt[:, :], hxbT_sb[:, :],
                                            huT_sb[:, u:u + 1], 0.0, ADD, MAX)
                                    else:
                                        nc.vector.tensor_scalar(
                                            rt[:, :], hxbT_sb[:, :],
                                            huT_sb[:, u:u + 1], 0.0, ADD, MAX)
                                    lhsT = w2big_sb[:, 32 * r:32 * r + 32]
                                    for h in range(2):
                                        nc.tensor.matmul(
                                            pbank[h][32 * j:32 * j + 32, :],
                                            lhsT,
                                            rt[:, h * HALF:(h + 1) * HALF],
                                            start=False,
                                            stop=(r == WAVES - 1),
                                            tile_position=(0, 32 * j),
                                            skip_group_check=True)
                            for h in range(2):
                                ev = evpool.tile([H, HALF], F32,
                                                 name=f"rep{rep}_evv_{co}_{h}",
                                                 tag="ev")
                                if h == 0:
                                    nc.vector.tensor_copy(ev[:, :],
                                                          pbank[h][:, :])
                                else:
                                    nc.scalar.copy(ev[:, :], pbank[h][:, :])
                                nc.sync.dma_start(
                                    logits_d[co * COHORT:(co + 1) * COHORT,
                                             h * HALF:(h + 1) * HALF],
                                    ev[:, :])
                elif DENSE_EV:
                    with tc.tile_pool(name=f"rep{rep}_plogd", bufs=4,
                                      space="PSUM") as plog:
                        zsl = w2big_sb[:, WAVES * 32:WAVES * 32 + 32]
                        for co in range(n_cohorts):
                            pbank = [plog.tile([H, HALF], F32,
                                               name=f"rep{rep}_plogd_{co}_{h}",
                                               tag="plogd") for h in range(2)]
                            for j in range(GROUPS):
                                for h in range(2):
                                    nc.tensor.matmul(
                                        pbank[h][32 * j:32 * j + 32, :], zsl,
                                        hxbT_sb[:, h * HALF:(h + 1) * HALF],
                                        start=True, stop=False,
                                        tile_position=(0, 32 * j),
                                        skip_group_check=True)
                            for r in range(WAVES):
                                for j in range(GROUPS):
                                    ul = 32 * j + r
                                    u = co * COHORT + ul
                                    rt = rpool.tile([H, C], r_dt,
                                                    name=f"rep{rep}_R_{u}",
                                                    tag="R")
                                    if (ul * ACT_NUM) % ACT_DEN < ACT_NUM:
                                        nc.scalar.activation(
                                            rt[:, :], hxbT_sb[:, :], RELU,
                                            bias=huT_sb[:, u:u + 1], scale=1.0)
                                    else:
                                        nc.vector.tensor_scalar(
                                            rt[:, :], hxbT_sb[:, :],
                                            huT_sb[:, u:u + 1], 0.0, ADD, MAX)
                                    lhsT = w2big_sb[:, 32 * r:32 * r + 32]
                                    for h in range(2):
                                        nc.tensor.matmul(
                                            pbank[h][32 * j:32 * j + 32, :],
                                            lhsT,
                                            rt[:, h * HALF:(h + 1) * HALF],
                                            start=False,
                                            stop=(r == WAVES - 1),
                                            tile_position=(0, 32 * j),
                                            skip_group_check=True)
                            for h in range(2):
                                ev = evpool.tile([H, HALF], F32,
                                                 name=f"rep{rep}_evd_{co}_{h}",
                                                 tag="ev")
                                if EV_ENG == "act":
                                    nc.scalar.activation(
                                        ev[:, :], pbank[h][:, :],
                                        mybir.ActivationFunctionType.Identity,
                                        bias=b2col_sb[:, 0:1], scale=1.0)
                                else:
                                    nc.vector.tensor_scalar(
                                        ev[:, :], pbank[h][:, :],
                                        b2col_sb[:, 0:1], None, ADD)
                                nc.sync.dma_start(
                                    logits_d[co * COHORT:(co + 1) * COHORT,
                                             h * HALF:(h + 1) * HALF],
                                    ev[:, :])
                else:
                    rtst = None
                    if PROBE == "mm_only":
                        rtst = []
                        for i in range(8):
                            t = workp.tile([H, C], r_dt,
                                           name=f"rep{rep}_rtst{i}",
                                           tag=f"rtst{i}")
                            nc.sync.dma_start(t[:, :], xcgT_d[0:H, :])
                            rtst.append(t)
                    with tc.tile_pool(name=f"rep{rep}_plog", bufs=8, space="PSUM") as plog:
                        for co in range(n_cohorts):
                            pbank = None
                            if PROBE != "relu_only":
                                pbank = [[plog.tile([H, HALF], F32,
                                                    name=f"rep{rep}_plog_{co}_{j}_{h}", tag="plog")
                                          for h in range(2)] for j in range(GROUPS)]
                            for r in range(WAVES):
                                for j in range(GROUPS):
                                    ul = 32 * j + r          # partition within cohort
                                    u = co * COHORT + ul     # user index on this core
                                    if PROBE == "mm_only":
                                        rt = rtst[(r * GROUPS + j) % 8]
                                    else:
                                        rt = rpool.tile([H, C], r_dt,
                                                        name=f"rep{rep}_R_{u}",
                                                        tag="R")
                                    if PROBE != "mm_only":
                                        if (ul * ACT_NUM) % ACT_DEN < ACT_NUM:
                                            nc.scalar.activation(
                                                rt[:, :], hxbT_sb[:, :], RELU,
                                                bias=huT_sb[:, u:u + 1], scale=1.0)
                                        else:
                                            nc.vector.tensor_scalar(
                                                rt[:, :], hxbT_sb[:, :],
                                                huT_sb[:, u:u + 1], 0.0, ADD, MAX)
                                    lhsT = w2big_sb[:, 32 * r:32 * r + 32]
                                    for h in range(2):
                                        if PROBE == "relu_only":
                                            continue
                                        rhs = rt[:, h * HALF:(h + 1) * HALF]
                                        out = pbank[j][h][32 * j:32 * j + 32, :]
                                        if R_DT == "f32" and MV_F32R:
                                            lhsT_mm = lhsT.bitcast(mybir.dt.float32r)
                                            rhs_mm = rhs.bitcast(mybir.dt.float32r)
                                        else:
                                            lhsT_mm, rhs_mm = lhsT, rhs
                                        nc.tensor.matmul(out, lhsT_mm, rhs_mm,
                                                         start=(r == 0),
                                                         stop=(r == WAVES - 1),
                                                         tile_position=(0, 32 * j))
                            # evict: +b2, PSUM band -> SBUF tile -> one DMA per half
                            for h in range(2):
                                if PROBE == "relu_only":
                                    continue
                                ev = evpool.tile([H, HALF], F32, name=f"rep{rep}_ev_{co}_{h}",
                                                 tag="ev")
                                for j in range(GROUPS):
                                    src = pbank[j][h][32 * j:32 * j + 32, :]
                                    dst = ev[32 * j:32 * j + 32, :]
                                    bslice = b2col_sb[32 * j:32 * j + 32, 0:1]
                                    if EV_ENG == "act":
                                        nc.scalar.activation(
                                            dst, src,
                                            mybir.ActivationFunctionType.Identity,
                                            bias=bslice, scale=1.0)
                                    else:
                                        nc.vector.tensor_scalar(
                                            dst, src, bslice, None, ADD)
                                nc.sync.dma_start(
                                    logits_d[co * COHORT:(co + 1) * COHORT,
                                             h * HALF:(h + 1) * HALF],
                                    ev[:, :])

    nc.compile()
    _CACHE[key] = nc
    return nc


def prep_inputs(h_u_bar, item_content, cold_ids, Wi, bi, W1, b1, W2, b2):
    """Host-side shard/replicate prep. Returns per-core in_maps."""
    f32 = np.float32
    h_u_bar = np.asarray(h_u_bar, f32)
    item_content = np.asarray(item_content, f32)
    cold_ids = np.asarray(cold_ids)
    Wi = np.asarray(Wi, f32)
    bi = np.asarray(bi, f32)
    W1 = np.asarray(W1, f32)
    b1 = np.asarray(b1, f32)
    W2 = np.asarray(W2, f32)
    b2 = np.asarray(b2, f32)

    xcg = item_content[cold_ids]                       # (C, CD) gather
    xcgT = np.ascontiguousarray(xcg.T).astype(np.float16)
    wiT = np.ascontiguousarray(Wi.T).astype(np.float16)
    stg_np = np.float16 if MODE == "v2" else np.float32
    w1hT = np.ascontiguousarray(W1[:, :D].T).astype(stg_np)   # (D, H)
    w1xT = np.ascontiguousarray(W1[:, D:].T).astype(stg_np)   # (D, H)
    b1col = np.ascontiguousarray(b1[:, None])
    bicol = np.ascontiguousarray(bi[:, None])
    b2col = np.full((H, 1), b2[0], f32)

    w2_np = {"f32": np.float32, "f16": np.float16}[R_DT]
    w2big = np.zeros((H, (WAVES + 1) * 32), w2_np)
    for r in range(WAVES):
        w2big[:, 32 * r + r] = W2[0].astype(w2_np)

    common = dict(xcgT=xcgT, wiT=wiT, bicol=bicol, w1xT=w1xT, w1hT=w1hT,
                  b1col=b1col, w2big=w2big, b2col=b2col)
    if MODE == "v2":
        common["b2w"] = np.full((H, 32), b2[0] / H, w2_np)
        common["ones"] = np.ones((H, HALF), w2_np)
        common["bicol2"] = np.ascontiguousarray(
            np.concatenate([bi, bi])[:, None]).astype(f32)
        common["w1xT2"] = np.ascontiguousarray(
            np.concatenate([w1xT, w1xT], axis=0)).astype(stg_np)
    in_maps = []
    for c in range(N_CORES):
        huT = np.ascontiguousarray(
            h_u_bar[c * UL:(c + 1) * UL].T).astype(stg_np)  # (D, UL)
        in_maps.append(dict(common, huT=huT))
    return in_maps


LAST_RESULTS = None  # BassKernelResults of the most recent run (for test.py)


def kernel(h_u_bar, item_content, cold_ids, Wi, bi, W1, b1, W2, b2,
           trace=False, trace_kwargs=None):
    global LAST_RESULTS
    from concourse.bass_utils import run_bass_kernel_spmd

    nc = build_bass()
    in_maps = prep_inputs(h_u_bar, item_content, cold_ids, Wi, bi, W1, b1,
                          W2, b2)
    kw = {}
    if trace:
        kw["trace"] = True
        if trace_kwargs:
            kw.update(trace_kwargs)
    res = run_bass_kernel_spmd(nc, in_maps, core_ids=list(range(N_CORES)), **kw)
    LAST_RESULTS = res
    out = np.concatenate([res.results[c]["logits"] for c in range(N_CORES)],
                         axis=0)
    return out.astype(np.float32)

